# revision 9
# baseline (speedup 1.0000x reference)
"""Chamfer distance kernel for Trainium2 (8 NeuronCores, Bass/Tile).

Strategy
--------
dist2[b, i, j] = ||targets[b,i] - preds[b,j]||^2 is computed on the tensor
engine with a K=9 "homogeneous coordinate" encoding:

    d2 = sum_d (t_d^2 * 1  +  t_d * (-2 p_d)  +  1 * p_d^2)

so a single matmul with contraction K=9 produces squared distances directly
in PSUM (fp32).  ScalarE drains PSUM -> SBUF fp16; VectorE accumulates
row-minima (over preds) and column-minima (over targets) with 2x-rate fp16
tensor_tensor(min); the final 128-partition fold for column minima uses
xbar DMA transposes + one batched reduce.  sqrt + means + cross-core
combining happen on the host (O(N) work only; all O(N^2) stays on device).

Sharding: 8 cores = 4 batches x 2 target-halves.  Each core computes its
2048 x 4096 block of the distance matrix: row-mins are complete per core;
col-mins are partial (its target half) and the two halves are min-combined
on the host.
"""

import sys

sys.path.insert(0, "/opt/trn_rl_repo")

import numpy as np

import concourse.bass as bass
import concourse.bacc as bacc
import concourse.tile as tile
from concourse import mybir

B, N, D = 4, 4096, 3
NCORES = 8
HALF = N // 2          # targets per core
NIT = HALF // 128      # 16 i-tiles of 128 rows
NTB = N // 128         # 32 col-transpose blocks total (16 per j-half)
K = 9                  # homogeneous encoding dim

F32 = mybir.dt.float32
F16 = mybir.dt.float16
BIG = 60000.0          # min-accumulator init (fits fp16; > any d2 here)


def _chamfer_tile_kernel(tc, rowmin, colmin, tpq, repeat=1):
    from contextlib import ExitStack

    nc = tc.nc
    MN = mybir.AluOpType.min

    with ExitStack() as ctx:
        consts = ctx.enter_context(tc.tile_pool(name="consts", bufs=1))
        accs = ctx.enter_context(tc.tile_pool(name="accs", bufs=1))
        drains = ctx.enter_context(tc.tile_pool(name="drains", bufs=2))
        psums = ctx.enter_context(tc.tile_pool(name="psums", bufs=2, space="PSUM"))
        outsp = ctx.enter_context(tc.tile_pool(name="outsp", bufs=1))

        # packed operands: [:, :HALF] = targets enc, [:, HALF:] = preds enc
        tpq_s = consts.tile([K, HALF + N], F32, tag="tpq")
        nc.sync.dma_start(out=tpq_s[:], in_=tpq)
        tq_s = tpq_s[:, :HALF]
        pq_s = tpq_s[:, HALF:]

        rowmin_s = outsp.tile([128, NIT], F32, tag="rowmin")
        colmin_s = outsp.tile([128, NTB], F32, tag="colmin")

        for _rep in range(repeat):
            _emit_body(tc, ctx, accs, drains, psums, tq_s, pq_s,
                       rowmin_s, colmin_s, MN)

        nc.sync.dma_start(out=rowmin, in_=rowmin_s[:])
        nc.sync.dma_start(out=colmin, in_=colmin_s[:])


def _emit_body(tc, ctx, accs, drains, psums, tq_s, pq_s, rowmin_s, colmin_s, MN):
    nc = tc.nc
    if True:
        rowacc = accs.tile([128, NIT, 512], F16, tag="rowacc")
        colacc = accs.tile([128, 2, 2048], F16, tag="colacc")
        colaccT = accs.tile([128, 2, 16, 128], F16, tag="colaccT")
        nc.vector.memset(rowacc[:], BIG)
        nc.vector.memset(colacc[:], BIG)

        for jo in range(2):          # j-half: preds [jo*2048, (jo+1)*2048)
            for g in range(4):       # group of 4 i-tiles
                d4 = drains.tile([128, 4, 2048], F16, tag="d4")
                for itg in range(4):
                    it = 4 * g + itg
                    ps = psums.tile([128, 2048], F32, tag="ps")
                    for jtl in range(4):
                        j0 = jo * 2048 + jtl * 512
                        nc.tensor.matmul(
                            ps[:, jtl * 512:(jtl + 1) * 512],
                            tq_s[:, it * 128:(it + 1) * 128],
                            pq_s[:, j0:j0 + 512],
                            start=True,
                            stop=True,
                        )
                    # drain 4 PSUM banks -> SBUF fp16 in one ACT op
                    nc.scalar.copy(d4[:, itg, :], ps[:])
                # row-min accumulate (over this j-half), 4 i-tiles per op
                for jtl in range(4):
                    nc.vector.tensor_tensor(
                        rowacc[:, 4 * g:4 * g + 4, :],
                        rowacc[:, 4 * g:4 * g + 4, :],
                        d4[:, :, jtl * 512:(jtl + 1) * 512],
                        MN,
                    )
                # col-min accumulate (over the 4 i-tiles)
                for itg in range(4):
                    nc.vector.tensor_tensor(
                        colacc[:, jo, :], colacc[:, jo, :], d4[:, itg, :], MN
                    )
            # fold col-min over the 128 partitions: xbar transpose + reduce
            for tb in range(16):
                nc.sync.dma_start_transpose(
                    colaccT[:, jo, tb, :],
                    colacc[:, jo, tb * 128:(tb + 1) * 128],
                )
            nc.vector.tensor_reduce(
                colmin_s[:, jo * 16:(jo + 1) * 16],
                colaccT[:, jo, :, :],
                axis=mybir.AxisListType.X,
                op=MN,
            )
        for g in range(4):
            nc.vector.tensor_reduce(
                rowmin_s[:, 4 * g:4 * g + 4],
                rowacc[:, 4 * g:4 * g + 4, :],
                axis=mybir.AxisListType.X,
                op=MN,
            )


_PROGRAMS = {}


def build_program(repeat=1):
    if repeat in _PROGRAMS:
        return _PROGRAMS[repeat]
    nc = bacc.Bacc("TRN2", target_bir_lowering=False, debug=False,
                   num_devices=NCORES)
    tpq = nc.dram_tensor("tpq", [K, HALF + N], F32, kind="ExternalInput").ap()
    rowmin = nc.dram_tensor("rowmin", [128, NIT], F32, kind="ExternalOutput").ap()
    colmin = nc.dram_tensor("colmin", [128, NTB], F32, kind="ExternalOutput").ap()
    with tile.TileContext(nc) as tc:
        _chamfer_tile_kernel(tc, rowmin, colmin, tpq, repeat=repeat)
    nc.compile()
    _PROGRAMS[repeat] = nc
    return nc


def make_in_maps(preds, targets):
    """Host-side shard + encode (O(N) prep only)."""
    preds = np.asarray(preds, dtype=np.float32)
    targets = np.asarray(targets, dtype=np.float32)
    in_maps = []
    for c in range(NCORES):
        b, h = divmod(c, 2)
        t = targets[b, h * HALF:(h + 1) * HALF]   # (2048, 3)
        p = preds[b]                              # (4096, 3)
        tpq = np.empty((K, HALF + N), np.float32)
        for d in range(D):
            tpq[3 * d + 0, :HALF] = t[:, d] * t[:, d]
            tpq[3 * d + 1, :HALF] = t[:, d]
            tpq[3 * d + 2, :HALF] = 1.0
            tpq[3 * d + 0, HALF:] = 1.0
            tpq[3 * d + 1, HALF:] = -2.0 * p[:, d]
            tpq[3 * d + 2, HALF:] = p[:, d] * p[:, d]
        in_maps.append({"tpq": tpq})
    return in_maps


def unshard(results):
    """Combine per-core row/col minima -> chamfer scalar (host, O(N))."""
    row_means = []
    col_halves = []
    for c in range(NCORES):
        rm = np.asarray(results[c]["rowmin"], np.float32).T.reshape(HALF)
        cm = np.asarray(results[c]["colmin"], np.float32).T.reshape(N)
        row_means.append(np.sqrt(np.maximum(rm, 0.0)))
        col_halves.append(cm)
    row_all = np.concatenate(row_means)           # 8 * 2048 = B*N target mins
    col_means = []
    for b in range(B):
        cm = np.minimum(col_halves[2 * b], col_halves[2 * b + 1])
        col_means.append(np.sqrt(np.maximum(cm, 0.0)))
    col_all = np.concatenate(col_means)           # B*N pred mins
    return np.float32(row_all.mean() + col_all.mean())


def run(preds, targets, trace=False, **kw):
    from concourse.bass_utils import run_bass_kernel_spmd

    nc = build_program()
    in_maps = make_in_maps(preds, targets)
    res = run_bass_kernel_spmd(nc, in_maps, list(range(NCORES)), trace=trace, **kw)
    return res


def kernel(preds, targets):
    res = run(preds, targets, trace=False)
    return unshard(res.results)


if __name__ == "__main__":
    rng = np.random.default_rng(0)
    p = rng.standard_normal((B, N, D), dtype=np.float32)
    t = rng.standard_normal((B, N, D), dtype=np.float32)
    out = kernel(p, t)
    print("kernel out:", out)


# revision 16
# speedup vs baseline: 1.6265x; 1.6265x over previous
"""Chamfer distance kernel for Trainium2 (8 NeuronCores, Bass/Tile).

Strategy
--------
dist2[b, i, j] = ||targets[b,i] - preds[b,j]||^2 is computed on the tensor
engine with a K=9 "homogeneous coordinate" encoding:

    d2 = sum_d (t_d^2 * 1  +  t_d * (-2 p_d)  +  1 * p_d^2)

so a single matmul with contraction K=9 produces squared distances directly
in PSUM (fp32).  ScalarE drains PSUM -> SBUF fp16; VectorE accumulates
row-minima (over preds) and column-minima (over targets) with 2x-rate fp16
tensor_tensor(min); the final 128-partition fold for column minima uses
xbar DMA transposes + one batched reduce.  sqrt + means + cross-core
combining happen on the host (O(N) work only; all O(N^2) stays on device).

Sharding: 8 cores = 4 batches x 2 target-halves.  Each core computes its
2048 x 4096 block of the distance matrix: row-mins are complete per core;
col-mins are partial (its target half) and the two halves are min-combined
on the host.
"""

import sys

sys.path.insert(0, "/opt/trn_rl_repo")

import numpy as np

import concourse.bass as bass
import concourse.bacc as bacc
import concourse.tile as tile
from concourse import mybir

B, N, D = 4, 4096, 3
NCORES = 8
HALF = N // 2          # targets per core
NIT = HALF // 128      # 16 i-tiles of 128 rows
NTB = N // 128         # 32 col-transpose blocks total (16 per j-half)
K = 9                  # homogeneous encoding dim

F32 = mybir.dt.float32
F16 = mybir.dt.float16
BIG = 60000.0          # min-accumulator init (fits fp16; > any d2 here)


ALL_FEATS = frozenset({"mm", "drain", "row", "col", "finale"})


def _chamfer_tile_kernel(tc, rowmin, colmin, tpq, repeat=1, feats=ALL_FEATS):
    from contextlib import ExitStack

    nc = tc.nc
    MN = mybir.AluOpType.min

    with ExitStack() as ctx:
        consts = ctx.enter_context(tc.tile_pool(name="consts", bufs=1))
        accs = ctx.enter_context(tc.tile_pool(name="accs", bufs=1))
        drains = ctx.enter_context(tc.tile_pool(name="drains", bufs=2))
        psums = ctx.enter_context(tc.tile_pool(name="psums", bufs=2, space="PSUM"))
        outsp = ctx.enter_context(tc.tile_pool(name="outsp", bufs=1))

        # packed operands: [:, :HALF] = targets enc, [:, HALF:] = preds enc
        tpq_s = consts.tile([K, HALF + N], F32, tag="tpq")
        nc.sync.dma_start(out=tpq_s[:], in_=tpq)
        tq_s = tpq_s[:, :HALF]
        pq_s = tpq_s[:, HALF:]

        rowmin_s = outsp.tile([128, NIT], F32, tag="rowmin")
        colmin_s = outsp.tile([128, NTB], F32, tag="colmin")
        if feats != ALL_FEATS:   # ablation variants may never write these
            nc.vector.memset(rowmin_s[:], 0.0)
            nc.vector.memset(colmin_s[:], 0.0)

        for _rep in range(repeat):
            _emit_body(tc, ctx, accs, drains, psums, tq_s, pq_s,
                       rowmin_s, colmin_s, MN, feats)

        nc.sync.dma_start(out=rowmin, in_=rowmin_s[:])
        nc.sync.dma_start(out=colmin, in_=colmin_s[:])


def _emit_body(tc, ctx, accs, drains, psums, tq_s, pq_s, rowmin_s, colmin_s, MN,
               feats=ALL_FEATS):
    nc = tc.nc
    if True:
        rowparts = accs.tile([128, NIT, 2], F32, tag="rowparts")
        colacc = accs.tile([128, 2, 2048], F16, tag="colacc")
        colaccT = accs.tile([128, 2, 16, 128], F16, tag="colaccT")
        nc.vector.memset(colacc[:], BIG)

        for jo in range(2):          # j-half: preds [jo*2048, (jo+1)*2048)
            for it in range(NIT):
                ps = psums.tile([128, 2048], F32, tag="ps")
                if "mm" in feats:
                    for jtl in range(4):
                        j0 = jo * 2048 + jtl * 512
                        nc.tensor.matmul(
                            ps[:, jtl * 512:(jtl + 1) * 512],
                            tq_s[:, it * 128:(it + 1) * 128],
                            pq_s[:, j0:j0 + 512],
                            start=True,
                            stop=True,
                        )
                # row-min of this tile (over its 2048 j's) straight from PSUM
                if "row" in feats:
                    nc.vector.tensor_reduce(
                        rowparts[:, it, jo:jo + 1],
                        ps[:],
                        axis=mybir.AxisListType.X,
                        op=MN,
                    )
                # col-min accumulate straight from PSUM (fp32 -> fp16 acc)
                if "col" in feats:
                    nc.vector.tensor_tensor(
                        colacc[:, jo, :], colacc[:, jo, :], ps[:], MN
                    )
            # fold col-min over the 128 partitions: xbar transpose + reduce
            if "finale" in feats:
                for tb in range(16):
                    nc.sync.dma_start_transpose(
                        colaccT[:, jo, tb, :],
                        colacc[:, jo, tb * 128:(tb + 1) * 128],
                    )
                nc.vector.tensor_reduce(
                    colmin_s[:, jo * 16:(jo + 1) * 16],
                    colaccT[:, jo, :, :],
                    axis=mybir.AxisListType.X,
                    op=MN,
                )
        if "finale" in feats:
            nc.vector.tensor_reduce(
                rowmin_s[:],
                rowparts[:],
                axis=mybir.AxisListType.X,
                op=MN,
            )


_PROGRAMS = {}


def build_program(repeat=1, feats=ALL_FEATS):
    key = (repeat, feats)
    if key in _PROGRAMS:
        return _PROGRAMS[key]
    nc = bacc.Bacc("TRN2", target_bir_lowering=False, debug=False,
                   num_devices=NCORES)
    tpq = nc.dram_tensor("tpq", [K, HALF + N], F32, kind="ExternalInput").ap()
    rowmin = nc.dram_tensor("rowmin", [128, NIT], F32, kind="ExternalOutput").ap()
    colmin = nc.dram_tensor("colmin", [128, NTB], F32, kind="ExternalOutput").ap()
    with tile.TileContext(nc) as tc:
        _chamfer_tile_kernel(tc, rowmin, colmin, tpq, repeat=repeat, feats=feats)
    nc.compile()
    _PROGRAMS[key] = nc
    return nc


def make_in_maps(preds, targets):
    """Host-side shard + encode (O(N) prep only)."""
    preds = np.asarray(preds, dtype=np.float32)
    targets = np.asarray(targets, dtype=np.float32)
    in_maps = []
    for c in range(NCORES):
        b, h = divmod(c, 2)
        t = targets[b, h * HALF:(h + 1) * HALF]   # (2048, 3)
        p = preds[b]                              # (4096, 3)
        tpq = np.empty((K, HALF + N), np.float32)
        for d in range(D):
            tpq[3 * d + 0, :HALF] = t[:, d] * t[:, d]
            tpq[3 * d + 1, :HALF] = t[:, d]
            tpq[3 * d + 2, :HALF] = 1.0
            tpq[3 * d + 0, HALF:] = 1.0
            tpq[3 * d + 1, HALF:] = -2.0 * p[:, d]
            tpq[3 * d + 2, HALF:] = p[:, d] * p[:, d]
        in_maps.append({"tpq": tpq})
    return in_maps


def unshard(results):
    """Combine per-core row/col minima -> chamfer scalar (host, O(N))."""
    row_means = []
    col_halves = []
    for c in range(NCORES):
        rm = np.asarray(results[c]["rowmin"], np.float32).T.reshape(HALF)
        cm = np.asarray(results[c]["colmin"], np.float32).T.reshape(N)
        row_means.append(np.sqrt(np.maximum(rm, 0.0)))
        col_halves.append(cm)
    row_all = np.concatenate(row_means)           # 8 * 2048 = B*N target mins
    col_means = []
    for b in range(B):
        cm = np.minimum(col_halves[2 * b], col_halves[2 * b + 1])
        col_means.append(np.sqrt(np.maximum(cm, 0.0)))
    col_all = np.concatenate(col_means)           # B*N pred mins
    return np.float32(row_all.mean() + col_all.mean())


def run(preds, targets, trace=False, **kw):
    from concourse.bass_utils import run_bass_kernel_spmd

    nc = build_program()
    in_maps = make_in_maps(preds, targets)
    res = run_bass_kernel_spmd(nc, in_maps, list(range(NCORES)), trace=trace, **kw)
    return res


def kernel(preds, targets):
    res = run(preds, targets, trace=False)
    return unshard(res.results)


if __name__ == "__main__":
    rng = np.random.default_rng(0)
    p = rng.standard_normal((B, N, D), dtype=np.float32)
    t = rng.standard_normal((B, N, D), dtype=np.float32)
    out = kernel(p, t)
    print("kernel out:", out)


# revision 19
# speedup vs baseline: 2.0482x; 1.2593x over previous
"""Chamfer distance kernel for Trainium2 (8 NeuronCores, Bass/Tile).

Strategy
--------
dist2[b, i, j] = ||targets[b,i] - preds[b,j]||^2 is computed on the tensor
engine with a K=9 "homogeneous coordinate" encoding:

    d2 = sum_d (t_d^2 * 1  +  t_d * (-2 p_d)  +  1 * p_d^2)

so a single matmul with contraction K=9 produces squared distances directly
in PSUM (fp32).  ScalarE drains PSUM -> SBUF fp16; VectorE accumulates
row-minima (over preds) and column-minima (over targets) with 2x-rate fp16
tensor_tensor(min); the final 128-partition fold for column minima uses
xbar DMA transposes + one batched reduce.  sqrt + means + cross-core
combining happen on the host (O(N) work only; all O(N^2) stays on device).

Sharding: 8 cores = 4 batches x 2 target-halves.  Each core computes its
2048 x 4096 block of the distance matrix: row-mins are complete per core;
col-mins are partial (its target half) and the two halves are min-combined
on the host.
"""

import sys

sys.path.insert(0, "/opt/trn_rl_repo")

import numpy as np

import concourse.bass as bass
import concourse.bacc as bacc
import concourse.tile as tile
from concourse import mybir

B, N, D = 4, 4096, 3
NCORES = 8
HALF = N // 2          # targets per core
NIT = HALF // 128      # 16 i-tiles of 128 rows
NTB = N // 128         # 32 col-transpose blocks total (16 per j-half)
K = 9                  # homogeneous encoding dim

F32 = mybir.dt.float32
F16 = mybir.dt.float16
BIG = 60000.0          # min-accumulator init (fits fp16; > any d2 here)


ALL_FEATS = frozenset({"mm", "drain", "row", "col", "finale"})


def _chamfer_tile_kernel(tc, rowmin, colmin, tpq, repeat=1, feats=ALL_FEATS):
    from contextlib import ExitStack

    nc = tc.nc
    MN = mybir.AluOpType.min

    with ExitStack() as ctx:
        consts = ctx.enter_context(tc.tile_pool(name="consts", bufs=1))
        accs = ctx.enter_context(tc.tile_pool(name="accs", bufs=1))
        drains = ctx.enter_context(tc.tile_pool(name="drains", bufs=2))
        psums = ctx.enter_context(tc.tile_pool(name="psums", bufs=2, space="PSUM"))
        outsp = ctx.enter_context(tc.tile_pool(name="outsp", bufs=1))

        # packed operands: [:, :HALF] = targets enc, [:, HALF:] = preds enc
        tpq_s = consts.tile([K, HALF + N], F32, tag="tpq")
        nc.sync.dma_start(out=tpq_s[:], in_=tpq)
        tq_s = tpq_s[:, :HALF]
        pq_s = tpq_s[:, HALF:]

        rowmin_s = outsp.tile([128, NIT], F32, tag="rowmin")
        colmin_s = outsp.tile([128, NTB], F32, tag="colmin")
        if feats != ALL_FEATS:   # ablation variants may never write these
            nc.vector.memset(rowmin_s[:], 0.0)
            nc.vector.memset(colmin_s[:], 0.0)

        for _rep in range(repeat):
            _emit_body(tc, ctx, accs, drains, psums, tq_s, pq_s,
                       rowmin_s, colmin_s, MN, feats)

        nc.sync.dma_start(out=rowmin, in_=rowmin_s[:])
        nc.sync.dma_start(out=colmin, in_=colmin_s[:])


def _emit_body(tc, ctx, accs, drains, psums, tq_s, pq_s, rowmin_s, colmin_s, MN,
               feats=ALL_FEATS):
    nc = tc.nc
    if True:
        rowparts = accs.tile([128, NIT, 2], F32, tag="rowparts")
        colacc = accs.tile([128, 2, 2048], F16, tag="colacc")
        colaccT = accs.tile([128, 2, 16, 128], F16, tag="colaccT")
        nc.vector.memset(colacc[:], BIG)

        for jo in range(2):          # j-half: preds [jo*2048, (jo+1)*2048)
            for it in range(NIT):
                ps = psums.tile([128, 2048], F32, tag="ps")
                if "mmsmall" in feats:
                    for jtl in range(16):
                        j0 = jo * 2048 + jtl * 128
                        nc.tensor.matmul(
                            ps[:, jtl * 128:(jtl + 1) * 128],
                            tq_s[:, it * 128:(it + 1) * 128],
                            pq_s[:, j0:j0 + 128],
                            start=True,
                            stop=True,
                        )
                elif "mm" in feats:
                    for jtl in range(4):
                        j0 = jo * 2048 + jtl * 512
                        nc.tensor.matmul(
                            ps[:, jtl * 512:(jtl + 1) * 512],
                            tq_s[:, it * 128:(it + 1) * 128],
                            pq_s[:, j0:j0 + 512],
                            start=True,
                            stop=True,
                        )
                # row-min of this tile (over its 2048 j's) straight from PSUM
                if "row" in feats:
                    nc.vector.tensor_reduce(
                        rowparts[:, it, jo:jo + 1],
                        ps[:],
                        axis=mybir.AxisListType.X,
                        op=MN,
                    )
                # col-min accumulate straight from PSUM (fp32 -> fp16 acc)
                if "col" in feats:
                    nc.vector.tensor_tensor(
                        colacc[:, jo, :], colacc[:, jo, :], ps[:], MN
                    )
        # fold col-min over the 128 partitions: one blocked xbar transpose
        # (out[p, tb, q] = colacc[q, tb*128+p]) + one batched reduce
        if "finale" in feats:
            nc.sync.dma_start_transpose(
                colaccT[:].rearrange("p a b f -> p (a b) f"),
                colacc[:].rearrange("p a b -> p (a b)"),
            )
            nc.vector.tensor_reduce(
                colmin_s[:],
                colaccT[:],
                axis=mybir.AxisListType.X,
                op=MN,
            )
            nc.vector.tensor_reduce(
                rowmin_s[:],
                rowparts[:],
                axis=mybir.AxisListType.X,
                op=MN,
            )


_PROGRAMS = {}


def build_program(repeat=1, feats=ALL_FEATS):
    key = (repeat, feats)
    if key in _PROGRAMS:
        return _PROGRAMS[key]
    nc = bacc.Bacc("TRN2", target_bir_lowering=False, debug=False,
                   num_devices=NCORES)
    tpq = nc.dram_tensor("tpq", [K, HALF + N], F32, kind="ExternalInput").ap()
    rowmin = nc.dram_tensor("rowmin", [128, NIT], F32, kind="ExternalOutput").ap()
    colmin = nc.dram_tensor("colmin", [128, NTB], F32, kind="ExternalOutput").ap()
    with tile.TileContext(nc) as tc:
        _chamfer_tile_kernel(tc, rowmin, colmin, tpq, repeat=repeat, feats=feats)
    nc.compile()
    _PROGRAMS[key] = nc
    return nc


def make_in_maps(preds, targets):
    """Host-side shard + encode (O(N) prep only)."""
    preds = np.asarray(preds, dtype=np.float32)
    targets = np.asarray(targets, dtype=np.float32)
    in_maps = []
    for c in range(NCORES):
        b, h = divmod(c, 2)
        t = targets[b, h * HALF:(h + 1) * HALF]   # (2048, 3)
        p = preds[b]                              # (4096, 3)
        tpq = np.empty((K, HALF + N), np.float32)
        for d in range(D):
            tpq[3 * d + 0, :HALF] = t[:, d] * t[:, d]
            tpq[3 * d + 1, :HALF] = t[:, d]
            tpq[3 * d + 2, :HALF] = 1.0
            tpq[3 * d + 0, HALF:] = 1.0
            tpq[3 * d + 1, HALF:] = -2.0 * p[:, d]
            tpq[3 * d + 2, HALF:] = p[:, d] * p[:, d]
        in_maps.append({"tpq": tpq})
    return in_maps


def unshard(results):
    """Combine per-core row/col minima -> chamfer scalar (host, O(N))."""
    row_means = []
    col_halves = []
    for c in range(NCORES):
        rm = np.asarray(results[c]["rowmin"], np.float32).T.reshape(HALF)
        cm = np.asarray(results[c]["colmin"], np.float32).T.reshape(N)
        row_means.append(np.sqrt(np.maximum(rm, 0.0)))
        col_halves.append(cm)
    row_all = np.concatenate(row_means)           # 8 * 2048 = B*N target mins
    col_means = []
    for b in range(B):
        cm = np.minimum(col_halves[2 * b], col_halves[2 * b + 1])
        col_means.append(np.sqrt(np.maximum(cm, 0.0)))
    col_all = np.concatenate(col_means)           # B*N pred mins
    return np.float32(row_all.mean() + col_all.mean())


def run(preds, targets, trace=False, **kw):
    from concourse.bass_utils import run_bass_kernel_spmd

    nc = build_program()
    in_maps = make_in_maps(preds, targets)
    res = run_bass_kernel_spmd(nc, in_maps, list(range(NCORES)), trace=trace, **kw)
    return res


def kernel(preds, targets):
    res = run(preds, targets, trace=False)
    return unshard(res.results)


if __name__ == "__main__":
    rng = np.random.default_rng(0)
    p = rng.standard_normal((B, N, D), dtype=np.float32)
    t = rng.standard_normal((B, N, D), dtype=np.float32)
    out = kernel(p, t)
    print("kernel out:", out)
